# revision 28
# baseline (speedup 1.0000x reference)
"""Trainium2 Bass kernel for nn_Net_75282186764473.

Math: reference pat() returns zm + stop_gradient(ze - zm) which numerically
equals ze -- the forward pass is just 5 explicit-Euler steps of the
"experiment" dynamics per stage, twice:
    q' = p ; p' = sin(1.1 q) @ (c2q(C) + Qn - I) + e
With u = 1.1 q, g_n = sin(u_n) @ W + eb  (W, eb scaled by 1.1*DT^2):
    u1 = u0 (p0 = 0)  =>  g1 = g0
    u2 = u0 + g0 ; u3 = u0 + 3 g0 ; u5 = u0 + 7 g0 + 2 g2 + g3
so each stage needs only 3 sins (u0, u2, u3) and weight passes with
{W, 2W, 4W}.  The state u accumulates directly in a PSUM bank: an
identity matmul seeds u0, then scaled weight passes advance it through
u2 -> u3 -> u5.  The e-bias rides as an extra contraction row against a
constant ones row (K = 197 / 207).

sin args can exceed [-pi, pi] (the ACT table domain), so every sin input
is range-reduced with the single custom DVE op add_range_wrap
(y + 2pi*((y < -pi) - (y > pi))), which doubles as the PSUM->SBUF read of
the accumulated state.  Wrapping shifts states by multiples of 2pi, which
is invisible to every sin and to the output: the output rows are the 10
class nodes, which start at 0 and stay far inside [-pi, pi].

Sharding: pure batch data-parallel across 8 cores (8192 rows each); x is
pre-transposed AND pre-wrapped (1.1*x wrapped into [-pi,pi]) on the host,
node-major [196, B].  Output rows are nodes 192:206 (32-aligned partition
base); host keeps the last 10 and divides by 1.1.
"""

import ml_dtypes
import numpy as np

import concourse.bacc as bacc
import concourse.bass as bass
import concourse.mybir as mybir
import concourse.tile as tile
from concourse.bass_utils import run_bass_kernel_spmd

AF = mybir.ActivationFunctionType
F32 = mybir.dt.float32
BF16 = mybir.dt.bfloat16

N_CORES = 8
B = 65536
BC = B // N_CORES          # 8192 batch rows per core
D1 = 196                   # stage-1 nodes
D1E = 197                  # + bias row
D2 = 206                   # stage-2 nodes (+10 class)
D2E = 207
P = 128
D1B = D1 - P               # 68
D1KB = D1E - P             # 69
D2B = D2 - P               # 78
D2KB = D2E - P             # 79
NOUT = 10
BT = 512                   # batch tile (one PSUM bank of fp32)
SC = 1.1                   # sin argument scale (1 + eta)
DT = 0.5 / 5
DT2 = DT * DT
TWO_PI = float(2.0 * np.pi)
PI = float(np.pi)

TRACE = False              # set True (e.g. from test.py) to NTFF-profile
LAST_RESULTS = None        # BassKernelResults of the most recent run

_CACHE = {}


def _build_program(bc=BC, num_devices=N_CORES):
    ntiles = bc // BT
    nc = bacc.Bacc(
        "TRN2",
        target_bir_lowering=False,
        debug=False,
        num_devices=num_devices,
    )
    xh_d = nc.dram_tensor("xh", [D1, bc], BF16, kind="ExternalInput").ap()
    xl_d = nc.dram_tensor("xl", [D1, bc], BF16, kind="ExternalInput").ap()
    w1_d = nc.dram_tensor("w1", [D1E, D1], BF16, kind="ExternalInput").ap()
    w2_d = nc.dram_tensor("w2", [D1E, D1], BF16, kind="ExternalInput").ap()
    w4_d = nc.dram_tensor("w4", [D1E, D1], BF16, kind="ExternalInput").ap()
    v1_d = nc.dram_tensor("v1", [D2E, D2], BF16, kind="ExternalInput").ap()
    v2_d = nc.dram_tensor("v2", [D2E, D2], BF16, kind="ExternalInput").ap()
    v4_d = nc.dram_tensor("v4", [D2E, D2], BF16, kind="ExternalInput").ap()
    i1b_d = nc.dram_tensor("i1b", [P, P], BF16, kind="ExternalInput").ap()
    i1_d = nc.dram_tensor("i1", [P, P], F32, kind="ExternalInput").ap()
    i2_d = nc.dram_tensor("i2", [D2B, D2B], F32, kind="ExternalInput").ap()
    # rows = nodes 192:206 (14 rows, 32-aligned partition base)
    out_d = nc.dram_tensor("out", [14, bc], F32, kind="ExternalOutput").ap()

    with tile.TileContext(nc) as tc:
        with (
            tc.tile_pool(name="wts", bufs=1) as wp,
            tc.tile_pool(name="io", bufs=3) as io,
            tc.tile_pool(name="sq", bufs=3) as sq,
            tc.tile_pool(name="psA", bufs=4, space=bass.MemorySpace.PSUM) as psA,
            tc.tile_pool(name="psB", bufs=4, space=bass.MemorySpace.PSUM) as psB,
        ):
            def load_w(dram, rows, name):
                ta = wp.tile([P, dram.shape[1]], BF16, tag=name + "a")
                nc.sync.dma_start(ta[:], dram[0:P, :])
                tb = wp.tile([rows - P, dram.shape[1]], BF16, tag=name + "b")
                nc.sync.dma_start(tb[:], dram[P:rows, :])
                return ta, tb

            w1a, w1b = load_w(w1_d, D1E, "w1")
            w2a, w2b = load_w(w2_d, D1E, "w2")
            w4a, w4b = load_w(w4_d, D1E, "w4")
            v1a, v1b = load_w(v1_d, D2E, "v1")
            v2a, v2b = load_w(v2_d, D2E, "v2")
            v4a, v4b = load_w(v4_d, D2E, "v4")
            i1b = wp.tile([P, P], BF16, tag="i1b")
            nc.sync.dma_start(i1b[:], i1b_d[:])
            i1 = wp.tile([P, P], F32, tag="i1")
            nc.sync.dma_start(i1[:], i1_d[:])
            i2 = wp.tile([D2B, D2B], F32, tag="i2")
            nc.sync.dma_start(i2[:], i2_d[:])

            for t in range(ntiles):
                cs = slice(t * BT, (t + 1) * BT)

                def sin_pair(src_a, src_b, name, nb, kb):
                    """sin of an already-wrapped input; ones row at kb-1
                    feeds the folded bias matmul row."""
                    sa = sq.tile([P, BT], BF16, tag=name + "a")
                    sb = sq.tile([kb, BT], BF16, tag=name + "b")
                    nc.gpsimd.memset(sb[64:kb, :], 1.0)
                    nc.scalar.activation(sa[:], src_a[:], AF.Sin)
                    nc.scalar.activation(sb[0:nb, :], src_b[:], AF.Sin)
                    return sa, sb

                def wrap_pair(bank_a, bank_b, name, nb):
                    """PSUM state -> wrapped SBUF copy in [-pi, pi]."""
                    wa = sq.tile([P, BT], F32, tag="wr" + name + "a")
                    wb = sq.tile([nb, BT], F32, tag="wr" + name + "b")
                    nc.vector.add_range_wrap(wa[:], bank_a[:], 0.0, PI, TWO_PI)
                    nc.vector.add_range_wrap(wb[:], bank_b[:], 0.0, PI, TWO_PI)
                    return wa, wb

                # ---- stage 1: state accumulates in PSUM bank (pa, pb) ----
                qa = io.tile([P, BT], BF16, tag="qa")
                nc.sync.dma_start(qa[:], xh_d[0:P, cs])
                qb = io.tile([D1B, BT], BF16, tag="qb")
                nc.sync.dma_start(qb[:], xh_d[P:D1, cs])
                la = io.tile([P, BT], BF16, tag="la")
                nc.sync.dma_start(la[:], xl_d[0:P, cs])
                lb = io.tile([D1B, BT], BF16, tag="lb")
                nc.sync.dma_start(lb[:], xl_d[P:D1, cs])

                # reconstruct the fp32-accurate wrapped input for sin0
                x0a = sq.tile([P, BT], F32, tag="x0a")
                nc.vector.tensor_add(x0a[:], qa[:], la[:])
                x0b = sq.tile([D1B, BT], F32, tag="x0b")
                nc.vector.tensor_add(x0b[:], qb[:], lb[:])
                s0a, s0b = sin_pair(x0a, x0b, "s0", D1B, D1KB)
                pa = psA.tile([P, BT], F32, tag="A")
                pb = psB.tile([D2B, BT], F32, tag="B")

                def s1_pass(wta, wtb, ra, rb, last=False):
                    nc.tensor.matmul(pa[:], wta[:, 0:P], ra[:],
                                     start=False, stop=last,
                                     skip_group_check=True)
                    nc.tensor.matmul(pa[:], wtb[:, 0:P], rb[:],
                                     start=False, stop=last,
                                     skip_group_check=True)
                    nc.tensor.matmul(pb[0:D1B, :], wta[:, P:D1], ra[:],
                                     start=False, stop=last,
                                     skip_group_check=True)
                    nc.tensor.matmul(pb[0:D1B, :], wtb[:, P:D1], rb[:],
                                     start=False, stop=last,
                                     skip_group_check=True)

                # seed: u0 = xh + xl (wrapped 1.1x, exact to fp32)
                nc.tensor.matmul(pa[:], i1b[:], qa[:], start=True, stop=False,
                                 skip_group_check=True)
                nc.tensor.matmul(pa[:], i1b[:], la[:], start=False, stop=False,
                                 skip_group_check=True)
                nc.tensor.matmul(pb[0:D1B, :], i1b[0:D1B, 0:D1B], qb[:],
                                 start=True, stop=False,
                                 skip_group_check=True)
                nc.tensor.matmul(pb[0:D1B, :], i1b[0:D1B, 0:D1B], lb[:],
                                 start=False, stop=False,
                                 skip_group_check=True)
                s1_pass(w1a, w1b, s0a, s0b)            # u2 = u0 + g0
                m2a, m2b = wrap_pair(pa, pb[0:D1B, :], "2", D1B)
                s2a, s2b = sin_pair(m2a, m2b, "s2", D1B, D1KB)
                s1_pass(w2a, w2b, s0a, s0b)            # u3 = u2 + 2 g0
                m3a, m3b = wrap_pair(pa, pb[0:D1B, :], "3", D1B)
                s3a, s3b = sin_pair(m3a, m3b, "s3", D1B, D1KB)
                s1_pass(w4a, w4b, s0a, s0b)            # + 4 g0
                s1_pass(w2a, w2b, s2a, s2b)            # + 2 g2
                s1_pass(w1a, w1b, s3a, s3b, last=True)  # + g3 -> u5
                # z2 = wrapped u5, padded with 10 zero class rows
                z2a = sq.tile([P, BT], F32, tag="z2a")
                z2b = sq.tile([D2B, BT], F32, tag="z2b")
                nc.gpsimd.memset(z2b[64:D2B, :], 0.0)
                nc.vector.add_range_wrap(z2a[:], pa[:], 0.0, PI, TWO_PI)
                nc.vector.add_range_wrap(z2b[0:D1B, :], pb[0:D1B, :],
                                         0.0, PI, TWO_PI)

                # ---- stage 2: same scheme on 206 nodes (ya, yb) ----
                t0a, t0b = sin_pair(z2a, z2b, "t0", D2B, D2KB)
                ya = psA.tile([P, BT], F32, tag="A")
                yb = psB.tile([D2B, BT], F32, tag="B")

                def s2_pass(wta, wtb, ra, rb, last_a=False):
                    nc.tensor.matmul(ya[:], wta[:, 0:P], ra[:],
                                     start=False, stop=last_a,
                                     skip_group_check=True)
                    nc.tensor.matmul(ya[:], wtb[:, 0:P], rb[:],
                                     start=False, stop=last_a,
                                     skip_group_check=True)
                    nc.tensor.matmul(yb[:], wta[:, P:D2], ra[:],
                                     start=False, stop=False,
                                     skip_group_check=True)
                    nc.tensor.matmul(yb[:], wtb[:, P:D2], rb[:],
                                     start=False, stop=False,
                                     skip_group_check=True)

                def trim_pass(wta, wtb, ra, rb, last=False):
                    # only the B block matters from here on (output rows);
                    # same cost as a trimmed matmul (time is N-bound)
                    nc.tensor.matmul(yb[:], wta[:, P:D2], ra[:],
                                     start=False, stop=last,
                                     skip_group_check=True)
                    nc.tensor.matmul(yb[:], wtb[:, P:D2], rb[:],
                                     start=False, stop=last,
                                     skip_group_check=True)

                nc.tensor.matmul(ya[:], i1[:], z2a[:], start=True, stop=False,
                                 skip_group_check=True)
                nc.tensor.matmul(yb[:], i2[:], z2b[:], start=True, stop=False,
                                 skip_group_check=True)
                s2_pass(v1a, v1b, t0a, t0b)            # u2'
                n2a, n2b = wrap_pair(ya, yb[0:D2B, :], "2p", D2B)
                t2a, t2b = sin_pair(n2a, n2b, "t2", D2B, D2KB)
                s2_pass(v2a, v2b, t0a, t0b, last_a=True)  # u3'
                n3a, n3b = wrap_pair(ya, yb[0:D2B, :], "3p", D2B)
                t3a, t3b = sin_pair(n3a, n3b, "t3", D2B, D2KB)
                trim_pass(v4a, v4b, t0a, t0b)          # + 4 g0'
                trim_pass(v2a, v2b, t2a, t2b)          # + 2 g2'
                trim_pass(v1a, v1b, t3a, t3b, last=True)  # + g3' -> u5'
                outt = io.tile([D2B, BT], F32, tag="outt")
                nc.vector.tensor_copy(outt[64:D2B, :], yb[64:D2B, :])
                nc.sync.dma_start(out_d[:, cs], outt[64:D2B, :])

    nc.compile()
    return nc


def _c2q(C):
    Q = 0.5 * (C + C.T)
    d = -Q.sum(axis=0)
    Q = Q.copy()
    Q[np.diag_indices_from(Q)] = d
    return Q


def _host_weights(fc_w, fc_b, qn, dim):
    """SC*DT2-scaled dynamics matrix with the bias folded as a last row."""
    W = SC * DT2 * (_c2q(np.asarray(fc_w, np.float64))
                    + np.asarray(qn, np.float64) - np.eye(dim))
    eb = SC * DT2 * np.asarray(fc_b, np.float64)
    return np.concatenate([W, eb[None, :]], axis=0)


def kernel(x, fc1_w, fc1_b, fc2_w, fc2_b, output_fac,
           Q_noise_small, Q_noise_large):
    global LAST_RESULTS
    if "nc" not in _CACHE:
        _CACHE["nc"] = _build_program()
    nc = _CACHE["nc"]

    w1 = _host_weights(fc1_w, fc1_b, Q_noise_small, D1)
    v1 = _host_weights(fc2_w, fc2_b, Q_noise_large, D2)

    BF = ml_dtypes.bfloat16

    def bf(a):
        return np.ascontiguousarray(np.asarray(a, np.float32).astype(BF))

    # u0 = 1.1*x wrapped into [-pi, pi] (single-period wrap, |1.1 x| < 3pi),
    # split into bf16 hi + lo so the seed matmul is fp32-accurate
    u = SC * np.asarray(x, np.float64)
    u = u - TWO_PI * ((u > PI).astype(np.float64)
                      - (u < -PI).astype(np.float64))
    xt = np.asarray(u.T, np.float32)  # [D1, B]
    xh = xt.astype(BF)
    xl = (xt - xh.astype(np.float32)).astype(BF)

    common = {
        "w1": bf(w1), "w2": bf(2.0 * w1), "w4": bf(4.0 * w1),
        "v1": bf(v1), "v2": bf(2.0 * v1), "v4": bf(4.0 * v1),
        "i1b": np.eye(P, dtype=BF),
        "i1": np.eye(P, dtype=np.float32),
        "i2": np.eye(D2B, dtype=np.float32),
    }
    in_maps = []
    for c in range(N_CORES):
        m = dict(common)
        m["xh"] = np.ascontiguousarray(xh[:, c * BC:(c + 1) * BC])
        m["xl"] = np.ascontiguousarray(xl[:, c * BC:(c + 1) * BC])
        in_maps.append(m)

    res = None
    last_exc = None
    for _attempt in range(3):
        try:
            res = run_bass_kernel_spmd(
                nc, in_maps, core_ids=list(range(N_CORES)), trace=TRACE)
            break
        except Exception as e:  # transient NRT/device hiccups
            last_exc = e
    if res is None:
        raise last_exc
    LAST_RESULTS = res

    out = np.empty((B, NOUT), np.float32)
    for c in range(N_CORES):
        out[c * BC:(c + 1) * BC, :] = res.results[c]["out"][4:14, :].T
    fac = float(np.asarray(output_fac)) / SC
    out = out * np.float32(fac)
    return out
